# revision 3
# baseline (speedup 1.0000x reference)
"""Trainium2 Bass kernel for nn_MultiHeadAttention_65481071395029.

8-core SPMD: core c handles batch b=c//2 and heads h0=(c%2)*8 .. h0+8.
Math per core (S=1024, DK=64, 8 heads):
  q = query @ WqT/8 + bq/8        (transposed layout: [dk, s])
  k = key   @ WkT   + bk
  asprow_h = tanh(aw_h . k_h + bias_m)   with aw = (aspect @ WdT + bd) @ weight_m
  scores = q_h.T k_h + short + maskbias + asprow  (maskbias = -60000 where mask==0)
  out = softmax(scores, axis=-1)  computed as exp(scores)/rowsum (no max-subtract
  needed: unmasked scores are O(10), masked underflow to exactly 0)

All HBM IO in fp16; accumulation f32 in PSUM. short+maskbias is injected into
PSUM via identity matmul on PE, aspect row via rank-1 ones matmul; ACT does a
single exp pass with accumulated rowsum; DVE does reciprocal + 4x scale.
"""

import numpy as np
from contextlib import ExitStack

B, S, D, H, DK = 4, 1024, 1024, 16, 64
HPC = 8          # heads per core
NPAIR = HPC // 2 # head pairs per core
QTN = S // 128   # q tiles
NEG = -60000.0   # masked-score bias (exp underflows to exactly 0)
N_CORES = 8

_compiled = None


def _build():
    import concourse.bass as bass  # noqa: F401
    import concourse.tile as tile
    from concourse import bacc, mybir

    f16, f32 = mybir.dt.float16, mybir.dt.float32
    AF = mybir.ActivationFunctionType
    OP = mybir.AluOpType

    nc = bacc.Bacc("TRN2", target_bir_lowering=False, debug=False)

    qTe_d = nc.dram_tensor("qTe", [S + 1, S], f16, kind="ExternalInput")
    kTe_d = nc.dram_tensor("kTe", [S + 1, S], f16, kind="ExternalInput")
    wq_d = nc.dram_tensor("wq", [S + 1, HPC * DK], f16, kind="ExternalInput")
    wk_d = nc.dram_tensor("wk", [S + 1, HPC * DK], f16, kind="ExternalInput")
    aw2_d = nc.dram_tensor("aw2", [128, NPAIR], f16, kind="ExternalInput")
    mb_d = nc.dram_tensor("mb", [S, S], f16, kind="ExternalInput")
    short_d = nc.dram_tensor("shortp", [HPC, S, S], f16, kind="ExternalInput")
    id_d = nc.dram_tensor("ident", [128, 128], f16, kind="ExternalInput")
    bm_d = nc.dram_tensor("bm", [1, 1], f32, kind="ExternalInput")
    out_d = nc.dram_tensor("out", [HPC, S, S], f16, kind="ExternalOutput")

    with tile.TileContext(nc) as tc, ExitStack() as ctx:
        consts = ctx.enter_context(tc.tile_pool(name="consts", bufs=1))
        stp = ctx.enter_context(tc.tile_pool(name="short_in", bufs=3))
        smp = ctx.enter_context(tc.tile_pool(name="sm", bufs=3))
        ep = ctx.enter_context(tc.tile_pool(name="exp", bufs=3))
        opl = ctx.enter_context(tc.tile_pool(name="outt", bufs=3))
        rsp = ctx.enter_context(tc.tile_pool(name="rows", bufs=8))
        mainps = ctx.enter_context(tc.tile_pool(name="mainps", bufs=2, space="PSUM"))
        projps = ctx.enter_context(tc.tile_pool(name="projps", bufs=2, space="PSUM"))

        # ---- constant loads ----
        qt_sb = consts.tile([128, 9, S], f16, tag="qt_sb")
        kt_sb = consts.tile([128, 9, S], f16, tag="kt_sb")
        for x_sb, x_d in ((qt_sb, qTe_d), (kt_sb, kTe_d)):
            for kt in range(8):
                nc.sync.dma_start(x_sb[:, kt, :], x_d[kt * 128:(kt + 1) * 128, :])
            nc.sync.dma_start(x_sb[0:1, 8, :], x_d[S:S + 1, :])
        wq_sb = consts.tile([128, 9, HPC * DK], f16, tag="wq_sb")
        wk_sb = consts.tile([128, 9, HPC * DK], f16, tag="wk_sb")
        for w_sb, w_d in ((wq_sb, wq_d), (wk_sb, wk_d)):
            for kt in range(8):
                nc.sync.dma_start(w_sb[:, kt, :], w_d[kt * 128:(kt + 1) * 128, :])
            nc.sync.dma_start(w_sb[0:1, 8, :], w_d[S:S + 1, :])
        aw2_sb = consts.tile([128, NPAIR], f16, tag="aw2_sb")
        nc.sync.dma_start(aw2_sb[:], aw2_d[:])
        mb_sb = consts.tile([128, QTN, S], f16, tag="mb_sb")
        for qt in range(QTN):
            nc.sync.dma_start(mb_sb[:, qt, :], mb_d[qt * 128:(qt + 1) * 128, :])
        id_sb = consts.tile([128, 128], f16, tag="id_sb")
        nc.sync.dma_start(id_sb[:], id_d[:])
        bm_sb = consts.tile([1, 1], f32, tag="bm_sb")
        nc.sync.dma_start(bm_sb[:], bm_d[:])
        ones_col = consts.tile([1, 128], f16, tag="ones_col")
        nc.vector.memset(ones_col[:], 1.0)

        # ---- projections: per pair of heads -> [128(dk x2), S] fp16 tiles ----
        kpair = [consts.tile([128, S], f16, name=f"kp{p}", tag=f"kp{p}")
                 for p in range(NPAIR)]
        qpair = [consts.tile([128, S], f16, name=f"qp{p}", tag=f"qp{p}")
                 for p in range(NPAIR)]

        def proj(w_sb, x_sb, dst, pr):
            ps = projps.tile([128, S], f32, tag="proj_ps")
            for c in (0, 512):
                for kt in range(9):
                    if kt < 8:
                        lhsT = w_sb[:, kt, pr * 128:(pr + 1) * 128]
                        rhs = x_sb[:, kt, c:c + 512]
                    else:
                        lhsT = w_sb[0:1, 8, pr * 128:(pr + 1) * 128]
                        rhs = x_sb[0:1, 8, c:c + 512]
                    nc.tensor.matmul(ps[:, c:c + 512], lhsT, rhs,
                                     start=(kt == 0), stop=(kt == 8))
            nc.vector.tensor_copy(dst[:], ps[:])

        for pr in range(NPAIR):
            proj(wk_sb, kt_sb, kpair[pr], pr)
        for pr in range(NPAIR):
            proj(wq_sb, qt_sb, qpair[pr], pr)

        # ---- aspect rows: asprow[h] = tanh(aw_h . k_h + bias_m) ----
        asprow = consts.tile([1, HPC, S], f16, tag="asprow")
        for h in range(HPC):
            pr, hi = divmod(h, 2)
            rlo = 64 * hi
            aps = projps.tile([1, S], f32, tag="proj_ps")
            for c in (0, 512):
                nc.tensor.matmul(aps[0:1, c:c + 512],
                                 aw2_sb[rlo:rlo + 64, pr:pr + 1],
                                 kpair[pr][rlo:rlo + 64, c:c + 512],
                                 start=True, stop=True)
            nc.scalar.activation(asprow[0:1, h, :], aps[0:1, :], AF.Tanh,
                                 bias=bm_sb[0:1, 0:1])

        # ---- main loop ----
        for h in range(HPC):
            pr, hi = divmod(h, 2)
            rlo = 64 * hi
            for qt in range(QTN):
                st = stp.tile([128, S], f16, tag="st")
                nc.sync.dma_start(st[:], short_d[h, qt * 128:(qt + 1) * 128, :])
                sm = smp.tile([128, S], f16, tag="sm")
                nc.vector.tensor_add(sm[:], st[:], mb_sb[:, qt, :])
                ps = mainps.tile([128, S], f32, tag="main_ps")
                qsl = qpair[pr][rlo:rlo + 64, qt * 128:(qt + 1) * 128]
                for c in (0, 512):
                    nc.tensor.matmul(ps[:, c:c + 512], id_sb[:], sm[:, c:c + 512],
                                     start=True, stop=False)
                for c in (0, 512):
                    nc.tensor.matmul(ps[:, c:c + 512], ones_col[:],
                                     asprow[0:1, h, c:c + 512],
                                     start=False, stop=False)
                for c in (0, 512):
                    nc.tensor.matmul(ps[:, c:c + 512], qsl,
                                     kpair[pr][rlo:rlo + 64, c:c + 512],
                                     start=False, stop=True)
                e = ep.tile([128, S], f16, tag="e")
                rs = rsp.tile([128, 1], f32, tag="rs")
                nc.scalar.activation(e[:], ps[:], AF.Exp, accum_out=rs[:])
                rec = rsp.tile([128, 1], f32, tag="rec")
                nc.vector.reciprocal(rec[:], rs[:])
                o = opl.tile([128, S], f16, tag="o")
                nc.vector.tensor_scalar(o[:], e[:], rec[:], None, OP.mult)
                nc.sync.dma_start(out_d[h, qt * 128:(qt + 1) * 128, :], o[:])

    nc.compile()
    return nc


def _prep_inputs(query, key, mask, aspect, short, Wq, bq, Wk, bk, Wd, bd,
                 weight_m, bias_m):
    f16 = np.float16
    ones_row = np.ones((1, S), np.float32)
    # tiny aspect chain on host: asp[b] = aspect[b] @ Wd.T + bd ; aw = asp @ weight_m
    asp = aspect @ Wd.T + bd                      # [B, DK]
    aw = np.einsum('bc,hcd->bhd', asp, weight_m)  # [B, H, DK]
    ident = np.eye(128, dtype=f16)
    bm = np.asarray(bias_m, np.float32).reshape(1, 1)

    in_maps = []
    for c in range(N_CORES):
        b, g = divmod(c, 2)
        h0 = g * HPC
        sl = slice(h0 * DK, (h0 + HPC) * DK)
        qTe = np.concatenate([query[b].T, ones_row], 0).astype(f16)
        kTe = np.concatenate([key[b].T, ones_row], 0).astype(f16)
        wq = np.concatenate([Wq[sl].T, bq[None, sl]], 0).astype(np.float32)
        wq = (wq * 0.125).astype(f16)
        wk = np.concatenate([Wk[sl].T, bk[None, sl]], 0).astype(f16)
        aw2 = np.empty((128, NPAIR), f16)
        for p in range(NPAIR):
            aw2[0:64, p] = aw[b, h0 + 2 * p]
            aw2[64:128, p] = aw[b, h0 + 2 * p + 1]
        mb = np.where(mask[b] == 0, np.float32(NEG), np.float32(0)).astype(f16)
        shortp = np.ascontiguousarray(short[b, h0:h0 + HPC]).astype(f16)
        in_maps.append({
            "qTe": qTe, "kTe": kTe, "wq": wq, "wk": wk, "aw2": aw2,
            "mb": mb, "shortp": shortp, "ident": ident, "bm": bm,
        })
    return in_maps


def kernel(query, key, mask, aspect, short, Wq, bq, Wk, bk, Wd, bd,
           weight_m, bias_m):
    global _compiled
    from concourse.bass_utils import run_bass_kernel_spmd

    args = [np.asarray(a) for a in (query, key, mask, aspect, short,
                                    Wq, bq, Wk, bk, Wd, bd, weight_m, bias_m)]
    if _compiled is None:
        _compiled = _build()
    nc = _compiled
    in_maps = _prep_inputs(*args)
    res = run_bass_kernel_spmd(nc, in_maps, core_ids=list(range(N_CORES)))
    out = np.empty((B, H, S, S), np.float32)
    for c in range(N_CORES):
        b, g = divmod(c, 2)
        out[b, g * HPC:(g + 1) * HPC] = res.results[c]["out"].astype(np.float32)
    return out


# revision 4
# speedup vs baseline: 1.2818x; 1.2818x over previous
"""Trainium2 Bass kernel for nn_MultiHeadAttention_65481071395029.

8-core SPMD: core c handles batch b=c//2 and heads h0=(c%2)*8 .. h0+8.
Math per core (S=1024, DK=64, 8 heads):
  q = query @ WqT/8 + bq/8        (transposed layout: [dk, s])
  k = key   @ WkT   + bk
  asprow_h = tanh(aw_h . k_h + bias_m)   with aw = (aspect @ WdT + bd) @ weight_m
  scores = q_h.T k_h (+ ones x asprow via 65-row contraction) + short + maskbias
  out = softmax(scores, axis=-1) = exp(scores)/rowsum (no max-subtract needed:
  unmasked scores are O(10); masked entries sit at ~-60000 and underflow to 0)

Engine plan per (head, qtile) iteration over a [128,1024] score tile:
  PE:  2x identity-inject matmuls put sm = short+maskbias (bf16) into PSUM,
       2x QK matmuls (fp16, contraction 65 = dk+aspect row) accumulate on top
  ACT: one Exp pass PSUM->SBUF fp16 with accumulated rowsum
  DVE: reciprocal + 4x tensor_scalar scale;  DMA: 256KB in / 256KB out
Projections run as head-pair matmuls [128(2x dk), s] then are rearranged into
per-head [65, s] tiles (ones/aspect row appended) via on-chip DMA.
"""

import numpy as np
from contextlib import ExitStack

B, S, D, H, DK = 4, 1024, 1024, 16, 64
HPC = 8          # heads per core
NPAIR = HPC // 2
QTN = S // 128   # q tiles
NEG = -60000.0
N_CORES = 8

_compiled = None


def _build():
    import concourse.bass as bass  # noqa: F401
    import concourse.tile as tile
    from concourse import bacc, mybir

    f16, bf16, f32 = mybir.dt.float16, mybir.dt.bfloat16, mybir.dt.float32
    AF = mybir.ActivationFunctionType
    OP = mybir.AluOpType

    nc = bacc.Bacc("TRN2", target_bir_lowering=False, debug=False)

    qTe_d = nc.dram_tensor("qTe", [S + 1, S], f16, kind="ExternalInput")
    kTe_d = nc.dram_tensor("kTe", [S + 1, S], f16, kind="ExternalInput")
    wq_d = nc.dram_tensor("wq", [S + 1, HPC * DK], f16, kind="ExternalInput")
    wk_d = nc.dram_tensor("wk", [S + 1, HPC * DK], f16, kind="ExternalInput")
    aw2_d = nc.dram_tensor("aw2", [DK, HPC], f16, kind="ExternalInput")
    mb_d = nc.dram_tensor("mb", [S, S], bf16, kind="ExternalInput")
    short_d = nc.dram_tensor("shortp", [HPC, S, S], bf16, kind="ExternalInput")
    id_d = nc.dram_tensor("ident", [128, 128], bf16, kind="ExternalInput")
    bm_d = nc.dram_tensor("bm", [1, 1], f32, kind="ExternalInput")
    out_d = nc.dram_tensor("out", [HPC, S, S], f16, kind="ExternalOutput")

    with tile.TileContext(nc) as tc, ExitStack() as ctx:
        consts = ctx.enter_context(tc.tile_pool(name="consts", bufs=1))
        stp = ctx.enter_context(tc.tile_pool(name="short_in", bufs=4))
        smp = ctx.enter_context(tc.tile_pool(name="sm", bufs=4))
        ep = ctx.enter_context(tc.tile_pool(name="exp", bufs=4))
        opl = ctx.enter_context(tc.tile_pool(name="outt", bufs=4))
        rsp = ctx.enter_context(tc.tile_pool(name="rows", bufs=12))
        asps = ctx.enter_context(tc.tile_pool(name="asps", bufs=2))
        psp = ctx.enter_context(tc.tile_pool(name="ps", bufs=4, space="PSUM"))

        # ---- PE warmup: trip the HAM busy window while initial DMAs run ----
        wdum = consts.tile([128, 512], bf16, tag="wdum")
        nc.vector.memset(wdum[:], 0.0)
        wps = psp.tile([128, 512], f32, tag="ps", name="warm_ps")
        for _ in range(12):
            nc.tensor.matmul(wps[:], wdum[:, 0:128], wdum[:], start=True, stop=True)

        # ---- constant loads (K side first: K-proj -> aspect chain) ----
        kt_sb = consts.tile([128, 9, S], f16, tag="kt_sb")
        qt_sb = consts.tile([128, 9, S], f16, tag="qt_sb")
        wk_sb = consts.tile([128, 9, HPC * DK], f16, tag="wk_sb")
        wq_sb = consts.tile([128, 9, HPC * DK], f16, tag="wq_sb")
        for x_sb, x_d in ((kt_sb, kTe_d), (wk_sb, wk_d)):
            for kt in range(8):
                nc.sync.dma_start(x_sb[:, kt, :], x_d[kt * 128:(kt + 1) * 128, :])
            nc.sync.dma_start(x_sb[0:1, 8, :], x_d[S:S + 1, :])
        for x_sb, x_d in ((qt_sb, qTe_d), (wq_sb, wq_d)):
            for kt in range(8):
                nc.sync.dma_start(x_sb[:, kt, :], x_d[kt * 128:(kt + 1) * 128, :])
            nc.sync.dma_start(x_sb[0:1, 8, :], x_d[S:S + 1, :])
        aw2_sb = consts.tile([DK, HPC], f16, tag="aw2_sb")
        nc.sync.dma_start(aw2_sb[:], aw2_d[:])
        id_sb = consts.tile([128, 128], bf16, tag="id_sb")
        nc.sync.dma_start(id_sb[:], id_d[:])
        bm_sb = consts.tile([1, 1], f32, tag="bm_sb")
        nc.sync.dma_start(bm_sb[:], bm_d[:])
        mb_sb = consts.tile([128, QTN, S], bf16, tag="mb_sb")
        for qt in range(QTN):
            nc.sync.dma_start(mb_sb[:, qt, :], mb_d[qt * 128:(qt + 1) * 128, :])

        # per-head transposed projection tiles, 65th row = ones (Q) / aspect (K)
        q65 = [consts.tile([65, S], f16, name=f"q65_{h}", tag=f"q65_{h}")
               for h in range(HPC)]
        k65 = [consts.tile([65, S], f16, name=f"k65_{h}", tag=f"k65_{h}")
               for h in range(HPC)]
        pairt = consts.tile([128, S], f16, tag="pairt", bufs=2)

        def proj(w_sb, x_sb, dst65, pr):
            ps = psp.tile([128, S], f32, tag="ps", name=f"proj_ps_{pr}")
            # bank-interleaved accumulation: alternate 512-chunks
            for kt in range(9):
                for c in (0, 512):
                    if kt < 8:
                        lhsT = w_sb[:, kt, pr * 128:(pr + 1) * 128]
                        rhs = x_sb[:, kt, c:c + 512]
                    else:
                        lhsT = w_sb[0:1, 8, pr * 128:(pr + 1) * 128]
                        rhs = x_sb[0:1, 8, c:c + 512]
                    nc.tensor.matmul(ps[:, c:c + 512], lhsT, rhs,
                                     start=(kt == 0), stop=(kt == 8))
            pair = consts.tile([128, S], f16, name=f"pair_{pr}", tag="pairt",
                               bufs=2)
            nc.vector.tensor_copy(pair[:], ps[:])
            nc.sync.dma_start(dst65[0][0:64, :], pair[0:64, :])
            nc.sync.dma_start(dst65[1][0:64, :], pair[64:128, :])

        for pr in range(NPAIR):
            proj(wk_sb, kt_sb, (k65[2 * pr], k65[2 * pr + 1]), pr)

        # aspect rows: k65[h][64,:] = tanh(aw_h . k_h + bias_m)
        for h in range(HPC):
            aps = psp.tile([1, S], f32, tag="ps", name=f"asp_ps_{h}")
            for c in (0, 512):
                nc.tensor.matmul(aps[0:1, c:c + 512], aw2_sb[:, h:h + 1],
                                 k65[h][0:64, c:c + 512], start=True, stop=True)
            ascr = asps.tile([1, S], f16, tag="ascr", name=f"ascr_{h}")
            nc.scalar.activation(ascr[:], aps[0:1, :], AF.Tanh,
                                 bias=bm_sb[0:1, 0:1])
            nc.sync.dma_start(k65[h][64:65, :], ascr[:])

        for pr in range(NPAIR):
            proj(wq_sb, qt_sb, (q65[2 * pr], q65[2 * pr + 1]), pr)
        for h in range(HPC):
            nc.vector.memset(q65[h][64:65, :], 1.0)

        # ---- main loop ----
        for h in range(HPC):
            for qt in range(QTN):
                st = stp.tile([128, S], bf16, tag="st", name=f"st_{h}_{qt}")
                nc.sync.dma_start(st[:], short_d[h, qt * 128:(qt + 1) * 128, :])
                sm = smp.tile([128, S], bf16, tag="sm", name=f"sm_{h}_{qt}")
                nc.vector.tensor_add(sm[:], st[:], mb_sb[:, qt, :])
                ps = psp.tile([128, S], f32, tag="ps", name=f"ps_{h}_{qt}")
                qsl = q65[h][:, qt * 128:(qt + 1) * 128]
                nc.tensor.matmul(ps[:, 0:512], id_sb[:], sm[:, 0:512],
                                 start=True, stop=False)
                nc.tensor.matmul(ps[:, 512:1024], id_sb[:], sm[:, 512:1024],
                                 start=True, stop=False)
                nc.tensor.matmul(ps[:, 0:512], qsl, k65[h][:, 0:512],
                                 start=False, stop=True)
                nc.tensor.matmul(ps[:, 512:1024], qsl, k65[h][:, 512:1024],
                                 start=False, stop=True)
                e = ep.tile([128, S], f16, tag="e", name=f"e_{h}_{qt}")
                rs = rsp.tile([128, 1], f32, tag="rs", name=f"rs_{h}_{qt}")
                nc.scalar.activation(e[:], ps[:], AF.Exp, accum_out=rs[:])
                rec = rsp.tile([128, 1], f32, tag="rec", name=f"rec_{h}_{qt}")
                nc.vector.reciprocal(rec[:], rs[:])
                o = opl.tile([128, S], f16, tag="o", name=f"o_{h}_{qt}")
                nc.vector.tensor_scalar(o[:], e[:], rec[:], None, OP.mult)
                nc.sync.dma_start(out_d[h, qt * 128:(qt + 1) * 128, :], o[:])

    nc.compile()
    return nc


def _prep_inputs(query, key, mask, aspect, short, Wq, bq, Wk, bk, Wd, bd,
                 weight_m, bias_m):
    import ml_dtypes
    f16 = np.float16
    bf16 = ml_dtypes.bfloat16
    ones_row = np.ones((1, S), np.float32)
    asp = aspect @ Wd.T + bd                      # [B, DK]
    aw = np.einsum('bc,hcd->bhd', asp, weight_m)  # [B, H, DK]
    ident = np.eye(128, dtype=bf16)
    bm = np.asarray(bias_m, np.float32).reshape(1, 1)

    in_maps = []
    for c in range(N_CORES):
        b, g = divmod(c, 2)
        h0 = g * HPC
        sl = slice(h0 * DK, (h0 + HPC) * DK)
        qTe = np.concatenate([query[b].T, ones_row], 0).astype(f16)
        kTe = np.concatenate([key[b].T, ones_row], 0).astype(f16)
        wq = np.concatenate([Wq[sl].T, bq[None, sl]], 0).astype(np.float32)
        wq = (wq * 0.125).astype(f16)
        wk = np.concatenate([Wk[sl].T, bk[None, sl]], 0).astype(f16)
        aw2 = np.ascontiguousarray(aw[b, h0:h0 + HPC].T).astype(f16)  # [DK, HPC]
        mb = np.where(mask[b] == 0, np.float32(NEG), np.float32(0)).astype(bf16)
        shortp = np.ascontiguousarray(short[b, h0:h0 + HPC]).astype(bf16)
        in_maps.append({
            "qTe": qTe, "kTe": kTe, "wq": wq, "wk": wk, "aw2": aw2,
            "mb": mb, "shortp": shortp, "ident": ident, "bm": bm,
        })
    return in_maps


def kernel(query, key, mask, aspect, short, Wq, bq, Wk, bk, Wd, bd,
           weight_m, bias_m):
    global _compiled
    from concourse.bass_utils import run_bass_kernel_spmd

    args = [np.asarray(a) for a in (query, key, mask, aspect, short,
                                    Wq, bq, Wk, bk, Wd, bd, weight_m, bias_m)]
    if _compiled is None:
        _compiled = _build()
    nc = _compiled
    in_maps = _prep_inputs(*args)
    res = run_bass_kernel_spmd(nc, in_maps, core_ids=list(range(N_CORES)))
    out = np.empty((B, H, S, S), np.float32)
    for c in range(N_CORES):
        b, g = divmod(c, 2)
        out[b, g * HPC:(g + 1) * HPC] = res.results[c]["out"].astype(np.float32)
    return out


# revision 5
# speedup vs baseline: 1.3633x; 1.0636x over previous
"""Trainium2 Bass kernel for nn_MultiHeadAttention_65481071395029.

8-core SPMD: core c handles batch b=c//2 and heads h0=(c%2)*8 .. h0+8.
Math per core (S=1024, DK=64, 8 heads):
  q = query @ WqT/8 + bq/8        (transposed layout: [dk, s])
  k = key   @ WkT   + bk
  asprow_h = tanh(aw_h . k_h + bias_m)   with aw = (aspect @ WdT + bd) @ weight_m
  scores = q_h.T k_h (+ ones x asprow via 65-row contraction) + short + maskbias
  out = softmax(scores, axis=-1) = exp(scores)/rowsum (no max-subtract needed:
  unmasked scores are O(10); masked entries sit at ~-60000 and underflow to 0)

Engine plan per (head, qtile) iteration over a [128,1024] score tile:
  PE:  2x identity-inject matmuls put sm = short+maskbias (bf16) into PSUM,
       2x QK matmuls (fp16, contraction 65 = dk+aspect row) accumulate on top
  ACT: one Exp pass PSUM->SBUF fp16 with accumulated rowsum
  DVE: reciprocal + 4x tensor_scalar scale
  DMA: 1MB transfers (4 qtiles grouped); short-in on sync ring, out on scalar
       ring, consts on gpsimd ring.
Projections run as head-pair matmuls [128(2x dk), s] then are rearranged into
per-head [65, s] tiles (ones/aspect row appended) via on-chip DMA.
"""

import numpy as np
from contextlib import ExitStack

B, S, D, H, DK = 4, 1024, 1024, 16, 64
HPC = 8          # heads per core
NPAIR = HPC // 2
QTN = S // 128   # q tiles
QG = 4           # q tiles per DMA group (1MB transfers)
NGRP = QTN // QG
NEG = -60000.0
N_CORES = 8

_compiled = None


def _build():
    import concourse.bass as bass  # noqa: F401
    import concourse.tile as tile
    from concourse import bacc, mybir

    f16, bf16, f32 = mybir.dt.float16, mybir.dt.bfloat16, mybir.dt.float32
    AF = mybir.ActivationFunctionType
    OP = mybir.AluOpType

    nc = bacc.Bacc("TRN2", target_bir_lowering=False, debug=False)

    qTe_d = nc.dram_tensor("qTe", [S + 1, S], f16, kind="ExternalInput")
    kTe_d = nc.dram_tensor("kTe", [S + 1, S], f16, kind="ExternalInput")
    wq_d = nc.dram_tensor("wq", [S + 1, HPC * DK], f16, kind="ExternalInput")
    wk_d = nc.dram_tensor("wk", [S + 1, HPC * DK], f16, kind="ExternalInput")
    aw2_d = nc.dram_tensor("aw2", [DK, HPC], f16, kind="ExternalInput")
    mb_d = nc.dram_tensor("mb", [S, S], bf16, kind="ExternalInput")
    short_d = nc.dram_tensor("shortp", [HPC, S, S], bf16, kind="ExternalInput")
    id_d = nc.dram_tensor("ident", [128, 128], bf16, kind="ExternalInput")
    bm_d = nc.dram_tensor("bm", [1, 1], f32, kind="ExternalInput")
    out_d = nc.dram_tensor("out", [HPC, S, S], f16, kind="ExternalOutput")

    # partition-major views of the [S, S] planes: [128, 8, 1024]
    mb_v = mb_d[0:S, :].rearrange("(n p) d -> p n d", p=128)
    qTe_v = qTe_d[0:S, :].rearrange("(n p) d -> p n d", p=128)
    kTe_v = kTe_d[0:S, :].rearrange("(n p) d -> p n d", p=128)
    wq_v = wq_d[0:S, :].rearrange("(n p) d -> p n d", p=128)
    wk_v = wk_d[0:S, :].rearrange("(n p) d -> p n d", p=128)

    with tile.TileContext(nc) as tc, ExitStack() as ctx:
        consts = ctx.enter_context(tc.tile_pool(name="consts", bufs=1))
        stp = ctx.enter_context(tc.tile_pool(name="short_in", bufs=2))
        smp = ctx.enter_context(tc.tile_pool(name="sm", bufs=2))
        ep = ctx.enter_context(tc.tile_pool(name="exp", bufs=2))
        opl = ctx.enter_context(tc.tile_pool(name="outt", bufs=2))
        rsp = ctx.enter_context(tc.tile_pool(name="rows", bufs=6))
        asps = ctx.enter_context(tc.tile_pool(name="asps", bufs=2))
        psp = ctx.enter_context(tc.tile_pool(name="ps", bufs=4, space="PSUM"))

        # ---- PE warmup: trip the HAM busy window while initial DMAs run ----
        wdum = consts.tile([128, 512], bf16, tag="wdum")
        nc.vector.memset(wdum[:], 0.0)
        wps = psp.tile([128, 512], f32, tag="ps", name="warm_ps")
        for _ in range(12):
            nc.tensor.matmul(wps[:], wdum[:, 0:128], wdum[:], start=True, stop=True)

        # ---- constant loads (K side first: K-proj -> aspect chain) ----
        kt_sb = consts.tile([128, 9, S], f16, tag="kt_sb")
        qt_sb = consts.tile([128, 9, S], f16, tag="qt_sb")
        wk_sb = consts.tile([128, 9, HPC * DK], f16, tag="wk_sb")
        wq_sb = consts.tile([128, 9, HPC * DK], f16, tag="wq_sb")
        nc.gpsimd.dma_start(kt_sb[:, 0:8, :], kTe_v)
        nc.gpsimd.dma_start(kt_sb[0:1, 8, :], kTe_d[S:S + 1, :])
        nc.gpsimd.dma_start(wk_sb[:, 0:8, :], wk_v)
        nc.gpsimd.dma_start(wk_sb[0:1, 8, :], wk_d[S:S + 1, :])
        nc.gpsimd.dma_start(qt_sb[:, 0:8, :], qTe_v)
        nc.gpsimd.dma_start(qt_sb[0:1, 8, :], qTe_d[S:S + 1, :])
        nc.gpsimd.dma_start(wq_sb[:, 0:8, :], wq_v)
        nc.gpsimd.dma_start(wq_sb[0:1, 8, :], wq_d[S:S + 1, :])
        aw2_sb = consts.tile([DK, HPC], f16, tag="aw2_sb")
        nc.gpsimd.dma_start(aw2_sb[:], aw2_d[:])
        id_sb = consts.tile([128, 128], bf16, tag="id_sb")
        nc.gpsimd.dma_start(id_sb[:], id_d[:])
        bm_sb = consts.tile([1, 1], f32, tag="bm_sb")
        nc.gpsimd.dma_start(bm_sb[:], bm_d[:])
        mb_sb = consts.tile([128, QTN, S], bf16, tag="mb_sb")
        nc.gpsimd.dma_start(mb_sb[:], mb_v)

        # per-head transposed projection tiles, 65th row = ones (Q) / aspect (K)
        q65 = [consts.tile([65, S], f16, name=f"q65_{h}", tag=f"q65_{h}")
               for h in range(HPC)]
        k65 = [consts.tile([65, S], f16, name=f"k65_{h}", tag=f"k65_{h}")
               for h in range(HPC)]

        def proj(w_sb, x_sb, dst65, pr):
            ps = psp.tile([128, S], f32, tag="ps", name=f"proj_ps_{pr}")
            # bank-interleaved accumulation: alternate 512-chunks
            for kt in range(9):
                for c in (0, 512):
                    if kt < 8:
                        lhsT = w_sb[:, kt, pr * 128:(pr + 1) * 128]
                        rhs = x_sb[:, kt, c:c + 512]
                    else:
                        lhsT = w_sb[0:1, 8, pr * 128:(pr + 1) * 128]
                        rhs = x_sb[0:1, 8, c:c + 512]
                    nc.tensor.matmul(ps[:, c:c + 512], lhsT, rhs,
                                     start=(kt == 0), stop=(kt == 8))
            pair = consts.tile([128, S], f16, name=f"pair_{pr}", tag="pairt",
                               bufs=2)
            nc.vector.tensor_copy(pair[:], ps[:])
            nc.gpsimd.dma_start(dst65[0][0:64, :], pair[0:64, :])
            nc.gpsimd.dma_start(dst65[1][0:64, :], pair[64:128, :])

        for pr in range(NPAIR):
            proj(wk_sb, kt_sb, (k65[2 * pr], k65[2 * pr + 1]), pr)

        # aspect rows: k65[h][64,:] = tanh(aw_h . k_h + bias_m)
        for h in range(HPC):
            aps = psp.tile([1, S], f32, tag="ps", name=f"asp_ps_{h}")
            for c in (0, 512):
                nc.tensor.matmul(aps[0:1, c:c + 512], aw2_sb[:, h:h + 1],
                                 k65[h][0:64, c:c + 512], start=True, stop=True)
            ascr = asps.tile([1, S], f16, tag="ascr", name=f"ascr_{h}")
            nc.scalar.activation(ascr[:], aps[0:1, :], AF.Tanh,
                                 bias=bm_sb[0:1, 0:1])
            nc.gpsimd.dma_start(k65[h][64:65, :], ascr[:])

        for pr in range(NPAIR):
            proj(wq_sb, qt_sb, (q65[2 * pr], q65[2 * pr + 1]), pr)
        for h in range(HPC):
            nc.vector.memset(q65[h][64:65, :], 1.0)

        # ---- main loop: groups of 4 qtiles -> 1MB DMA transfers ----
        for h in range(HPC):
            short_v = short_d[h].rearrange("(n p) d -> p n d", p=128)
            out_v = out_d[h].rearrange("(n p) d -> p n d", p=128)
            for g in range(NGRP):
                q0 = g * QG
                st = stp.tile([128, QG, S], bf16, tag="st", name=f"st_{h}_{g}")
                nc.sync.dma_start(st[:], short_v[:, q0:q0 + QG, :])
                sm = smp.tile([128, QG, S], bf16, tag="sm", name=f"sm_{h}_{g}")
                nc.vector.tensor_add(sm[:], st[:], mb_sb[:, q0:q0 + QG, :])
                e = ep.tile([128, QG, S], f16, tag="e", name=f"e_{h}_{g}")
                rs = rsp.tile([128, QG], f32, tag="rs", name=f"rs_{h}_{g}")
                rec = rsp.tile([128, QG], f32, tag="rec", name=f"rec_{h}_{g}")
                o = opl.tile([128, QG, S], f16, tag="o", name=f"o_{h}_{g}")
                for j in range(QG):
                    qt = q0 + j
                    ps = psp.tile([128, S], f32, tag="ps", name=f"ps_{h}_{qt}")
                    qsl = q65[h][:, qt * 128:(qt + 1) * 128]
                    nc.tensor.matmul(ps[:, 0:512], id_sb[:], sm[:, j, 0:512],
                                     start=True, stop=False)
                    nc.tensor.matmul(ps[:, 512:1024], id_sb[:], sm[:, j, 512:1024],
                                     start=True, stop=False)
                    nc.tensor.matmul(ps[:, 0:512], qsl, k65[h][:, 0:512],
                                     start=False, stop=True)
                    nc.tensor.matmul(ps[:, 512:1024], qsl, k65[h][:, 512:1024],
                                     start=False, stop=True)
                    nc.scalar.activation(e[:, j, :], ps[:], AF.Exp,
                                         accum_out=rs[:, j:j + 1])
                nc.vector.reciprocal(rec[:], rs[:])
                for j in range(QG):
                    nc.vector.tensor_scalar(o[:, j, :], e[:, j, :],
                                            rec[:, j:j + 1], None, OP.mult)
                nc.scalar.dma_start(out_v[:, q0:q0 + QG, :], o[:])

    nc.compile()
    return nc


def _prep_inputs(query, key, mask, aspect, short, Wq, bq, Wk, bk, Wd, bd,
                 weight_m, bias_m):
    import ml_dtypes
    f16 = np.float16
    bf16 = ml_dtypes.bfloat16
    ones_row = np.ones((1, S), np.float32)
    asp = aspect @ Wd.T + bd                      # [B, DK]
    aw = np.einsum('bc,hcd->bhd', asp, weight_m)  # [B, H, DK]
    ident = np.eye(128, dtype=bf16)
    bm = np.asarray(bias_m, np.float32).reshape(1, 1)

    in_maps = []
    for c in range(N_CORES):
        b, g = divmod(c, 2)
        h0 = g * HPC
        sl = slice(h0 * DK, (h0 + HPC) * DK)
        qTe = np.concatenate([query[b].T, ones_row], 0).astype(f16)
        kTe = np.concatenate([key[b].T, ones_row], 0).astype(f16)
        wq = np.concatenate([Wq[sl].T, bq[None, sl]], 0).astype(np.float32)
        wq = (wq * 0.125).astype(f16)
        wk = np.concatenate([Wk[sl].T, bk[None, sl]], 0).astype(f16)
        aw2 = np.ascontiguousarray(aw[b, h0:h0 + HPC].T).astype(f16)  # [DK, HPC]
        mb = np.where(mask[b] == 0, np.float32(NEG), np.float32(0)).astype(bf16)
        shortp = np.ascontiguousarray(short[b, h0:h0 + HPC]).astype(bf16)
        in_maps.append({
            "qTe": qTe, "kTe": kTe, "wq": wq, "wk": wk, "aw2": aw2,
            "mb": mb, "shortp": shortp, "ident": ident, "bm": bm,
        })
    return in_maps


def kernel(query, key, mask, aspect, short, Wq, bq, Wk, bk, Wd, bd,
           weight_m, bias_m):
    global _compiled
    from concourse.bass_utils import run_bass_kernel_spmd

    args = [np.asarray(a) for a in (query, key, mask, aspect, short,
                                    Wq, bq, Wk, bk, Wd, bd, weight_m, bias_m)]
    if _compiled is None:
        _compiled = _build()
    nc = _compiled
    in_maps = _prep_inputs(*args)
    res = run_bass_kernel_spmd(nc, in_maps, core_ids=list(range(N_CORES)))
    out = np.empty((B, H, S, S), np.float32)
    for c in range(N_CORES):
        b, g = divmod(c, 2)
        out[b, g * HPC:(g + 1) * HPC] = res.results[c]["out"].astype(np.float32)
    return out
